# revision 4
# baseline (speedup 1.0000x reference)
"""VQ codebook kernel for 8 TRN2 NeuronCores.

Computation (matches the reference):
    projected = features @ W.T + b            [B, D]
    projected = l2_normalize(projected)        [B, D]
    sims      = projected @ prototypes.T       [B, K]
    act       = softmax(sims / T, axis=-1)     [B, K]
    best_idx  = argmax(sims, axis=-1)          [B]

Distribution: prototypes are sharded on the K axis across the 8 cores
(tensor-parallel codebook).  Every core computes the full projection
(cheap relative to the similarity matmul), its [B, K/8] similarity
block, exp() with the row 1/(T*norm) scale folded in, a local softmax
numerator + row-sum, then the row-sums are AllReduce'd across cores and
each core normalizes + writes its activation block.  Local (max, argmax)
per row are written out and combined on the host (a 16 KB gather).

Key trick: the projection rows are never normalized explicitly.  The
similarity matmul uses the raw projection, and exp((sims/norm)/T) is
computed by the scalar engine as exp(raw * s_row) with the per-row
scale s_row = 1/(T*norm_row) — argmax is invariant to the positive
per-row scale, so it can run on the exp values directly.

Layouts: the PE contracts along the partition axis, so the host passes
features.T [D, B], W.T [D, E] and the prototype shard transposed
[E, K/8].  Host-side transposes are pure data movement (no FLOPs).
"""

import numpy as np

import concourse.bass as bass
import concourse.bacc as bacc
import concourse.tile as tile
import concourse.mybir as mybir
from concourse import bass_utils

B = 1024          # batch
D = 1024          # feature dim (projection contraction)
E = 1024          # projection output dim (similarity contraction)
K = 32768         # prototypes
N_CORES = 8
KS = K // N_CORES  # 4096 prototypes per core
TEMP = 0.1

P = 128           # partitions
NBLK = 512        # matmul moving-operand width (fp32 max, one PSUM bank)
NB = B // P       # 8 batch tiles
NE = E // P       # 8 e tiles
ND = D // P       # 8 d tiles
NJ = KS // NBLK   # 8 k blocks per core
GB = 4            # batch tiles per group (SBUF budget for exp tiles)
NG = NB // GB     # 2 groups

F32 = mybir.dt.float32
U32 = mybir.dt.uint32

_CACHE = {}


def _build():
    if "nc" in _CACHE:
        return _CACHE["nc"]

    nc = bacc.Bacc("TRN2", target_bir_lowering=False, debug=False,
                   num_devices=N_CORES)

    featT = nc.dram_tensor("featT", [D, B], F32, kind="ExternalInput").ap()
    WT = nc.dram_tensor("WT", [D, E], F32, kind="ExternalInput").ap()
    bvec = nc.dram_tensor("bvec", [E], F32, kind="ExternalInput").ap()
    protT = nc.dram_tensor("protT", [E, KS], F32, kind="ExternalInput").ap()

    act = nc.dram_tensor("act", [B, KS], F32, kind="ExternalOutput").ap()
    lmax = nc.dram_tensor("lmax", [B, 8], F32, kind="ExternalOutput").ap()
    lidx = nc.dram_tensor("lidx", [B, 8], U32, kind="ExternalOutput").ap()

    AF = mybir.ActivationFunctionType
    ALU = mybir.AluOpType
    AX = mybir.AxisListType

    with tile.TileContext(nc) as tc:
        with tc.tile_pool(name="persist", bufs=1) as pp, \
             tc.tile_pool(name="psum", bufs=4, space="PSUM") as psum, \
             tc.tile_pool(name="psum_n", bufs=2, space="PSUM") as psum_n, \
             tc.tile_pool(name="dram", bufs=2, space="DRAM") as dram:

            btile = pp.tile([P, NE], F32, tag="btile", name="btile")
            nc.sync.dma_start(btile[:], bvec.rearrange("(t p) -> p t", p=P))
            ones = pp.tile([P, 1], F32, tag="ones", name="ones")
            nc.vector.memset(ones[:], 1.0)
            s_tile = pp.tile([P, NB], F32, tag="s_tile", name="s_tile")  # 1/(T*norm) per row
            projT_sb = [pp.tile([P, B], F32, tag=f"projT{e}", name=f"projT{e}") for e in range(NE)]

            # ---- phase 1: projection projT[e, b] = (features @ W.T + b).T
            with tc.tile_pool(name="inw", bufs=1) as p_in, \
                 tc.tile_pool(name="sq", bufs=3) as p_sq:
                featT_sb = []
                WT_sb = []
                for d in range(ND):
                    ft = p_in.tile([P, B], F32, tag=f"ft{d}", name=f"ft{d}")
                    nc.sync.dma_start(ft[:], featT[d * P:(d + 1) * P, :])
                    featT_sb.append(ft)
                    wt = p_in.tile([P, E], F32, tag=f"wt{d}", name=f"wt{d}")
                    nc.sync.dma_start(wt[:], WT[d * P:(d + 1) * P, :])
                    WT_sb.append(wt)

                for e in range(NE):
                    for h in range(B // NBLK):
                        ps = psum.tile([P, NBLK], F32, tag="ps", name="ps")
                        for d in range(ND):
                            nc.tensor.matmul(
                                ps[:],
                                lhsT=WT_sb[d][:, e * P:(e + 1) * P],
                                rhs=featT_sb[d][:, h * NBLK:(h + 1) * NBLK],
                                start=(d == 0), stop=(d == ND - 1))
                        nc.vector.tensor_scalar_add(
                            projT_sb[e][:, h * NBLK:(h + 1) * NBLK],
                            ps[:], btile[:, e:e + 1])

                # ---- phase 2: row norms -> s_row = 1/(T*norm_row)
                # norms^2 land directly in [b-partition, 1] layout via
                # matmul(lhsT=squares_tile, rhs=ones): out[b, 0] =
                # sum_e sq[e, b].
                sq_sb = []
                for e in range(NE):
                    sq = p_sq.tile([P, B], F32, tag=f"sqt{e}", name=f"sqt{e}",
                                   bufs=1)
                    nc.vector.tensor_mul(sq[:], projT_sb[e][:], projT_sb[e][:])
                    sq_sb.append(sq)
                for bt in range(NB):
                    pnb = psum_n.tile([P, 1], F32, tag="pnb", name="pnb")
                    for e in range(NE):
                        nc.tensor.matmul(
                            pnb[:], lhsT=sq_sb[e][:, bt * P:(bt + 1) * P],
                            rhs=ones[:], start=(e == 0), stop=(e == NE - 1))
                    rec = p_sq.tile([P, 1], F32, tag="rec", name="rec")
                    nc.vector.reciprocal(rec[:], pnb[:])
                    # sqrt((1/norm^2) * (1/T^2)) = 1/(T*norm)
                    nc.scalar.activation(s_tile[:, bt:bt + 1], rec[:], AF.Sqrt,
                                         scale=1.0 / (TEMP * TEMP))

            # ---- phase 3: similarities, exp, denominators, argmax
            with tc.tile_pool(name="sims", bufs=1) as p_sims, \
                 tc.tile_pool(name="pt", bufs=2) as p_pt, \
                 tc.tile_pool(name="small", bufs=2) as p_sm:
                for g in range(NG):
                    exp_t = [p_sims.tile([P, KS], F32, tag=f"exp{bl}", name=f"exp{bl}")
                             for bl in range(GB)]
                    dsum_t = [p_sm.tile([P, NJ], F32, tag=f"dsum{bl}", name=f"dsum{bl}")
                              for bl in range(GB)]
                    for j in range(NJ):
                        pts = []
                        for e in range(NE):
                            pt = p_pt.tile([P, NBLK], F32, tag=f"pt{e}", name=f"pt{e}")
                            nc.sync.dma_start(
                                pt[:],
                                protT[e * P:(e + 1) * P,
                                      j * NBLK:(j + 1) * NBLK])
                            pts.append(pt)
                        for bl in range(GB):
                            bt = g * GB + bl
                            ps = psum.tile([P, NBLK], F32, tag="ps", name="ps")
                            for e in range(NE):
                                nc.tensor.matmul(
                                    ps[:],
                                    lhsT=projT_sb[e][:, bt * P:(bt + 1) * P],
                                    rhs=pts[e][:],
                                    start=(e == 0), stop=(e == NE - 1))
                            nc.scalar.activation(
                                exp_t[bl][:, j * NBLK:(j + 1) * NBLK],
                                ps[:], AF.Exp,
                                scale=s_tile[:, bt:bt + 1],
                                accum_out=dsum_t[bl][:, j:j + 1])

                    den = p_sm.tile([P, GB], F32, tag="den", name="den")
                    for bl in range(GB):
                        nc.vector.tensor_reduce(
                            den[:, bl:bl + 1], dsum_t[bl][:],
                            axis=AX.X, op=ALU.add)
                    cin = dram.tile([P, GB], F32, tag="cin", name="cin")
                    cout = dram.tile([P, GB], F32, tag="cout", name="cout")
                    nc.sync.dma_start(cin[:], den[:])
                    nc.gpsimd.collective_compute(
                        "AllReduce", ALU.add,
                        replica_groups=[list(range(N_CORES))],
                        ins=[cin.opt()], outs=[cout.opt()])
                    gd = p_sm.tile([P, GB], F32, tag="gd", name="gd")
                    nc.sync.dma_start(gd[:], cout[:])
                    rd = p_sm.tile([P, GB], F32, tag="rd", name="rd")
                    nc.vector.reciprocal(rd[:], gd[:])

                    for bl in range(GB):
                        bt = g * GB + bl
                        nc.vector.tensor_scalar_mul(
                            exp_t[bl][:], exp_t[bl][:], rd[:, bl:bl + 1])
                        m8 = p_sm.tile([P, 8], F32, tag="m8", name="m8")
                        i8 = p_sm.tile([P, 8], U32, tag="i8", name="i8")
                        nc.vector.max(m8[:], exp_t[bl][:])
                        nc.vector.max_index(i8[:], m8[:], exp_t[bl][:])
                        nc.sync.dma_start(act[bt * P:(bt + 1) * P, :],
                                          exp_t[bl][:])
                        nc.sync.dma_start(lmax[bt * P:(bt + 1) * P, :], m8[:])
                        nc.sync.dma_start(lidx[bt * P:(bt + 1) * P, :], i8[:])

    nc.compile()
    _CACHE["nc"] = nc
    return nc


def kernel(features, W, b, prototypes, _run_kwargs=None):
    nc = _build()

    featT = np.ascontiguousarray(np.asarray(features, dtype=np.float32).T)
    WT = np.ascontiguousarray(np.asarray(W, dtype=np.float32).T)
    bvec = np.asarray(b, dtype=np.float32)
    prototypes = np.asarray(prototypes, dtype=np.float32)

    in_maps = []
    for c in range(N_CORES):
        shard = np.ascontiguousarray(prototypes[c * KS:(c + 1) * KS].T)
        in_maps.append({"featT": featT, "WT": WT, "bvec": bvec,
                        "protT": shard})

    res = bass_utils.run_bass_kernel_spmd(
        nc, in_maps, core_ids=list(range(N_CORES)), **(_run_kwargs or {}))
    if _run_kwargs:
        _CACHE["last_result"] = res

    act = np.concatenate([res.results[c]["act"] for c in range(N_CORES)],
                         axis=1)
    lmax = np.stack([res.results[c]["lmax"][:, 0] for c in range(N_CORES)])
    lidx = np.stack([res.results[c]["lidx"][:, 0] for c in range(N_CORES)])
    best_core = np.argmax(lmax, axis=0)                       # [B]
    rows = np.arange(B)
    best_idx = (best_core * KS + lidx[best_core, rows]).astype(np.int32)
    return act, best_idx


# revision 5
# speedup vs baseline: 1.2972x; 1.2972x over previous
"""VQ codebook kernel for 8 TRN2 NeuronCores.

Computation (matches the reference):
    projected = features @ W.T + b            [B, D]
    projected = l2_normalize(projected)        [B, D]
    sims      = projected @ prototypes.T       [B, K]
    act       = softmax(sims / T, axis=-1)     [B, K]
    best_idx  = argmax(sims, axis=-1)          [B]

Distribution: prototypes are sharded on the K axis across the 8 cores
(tensor-parallel codebook).  Every core computes the full projection
(cheap relative to the similarity matmul), its [B, K/8] similarity
block, exp() with the row 1/(T*norm) scale folded in, a local softmax
numerator + row-sum, then the row-sums are AllReduce'd across cores and
each core normalizes + writes its activation block.  Local (max, argmax)
per row are written out and combined on the host (a 16 KB gather).

Precision/speed: fp32 matmuls on the PE cost 4 cycles/row (two
half-speed passes + per-pass weight reloads).  Instead every matmul
runs as a 3-pass bf16 decomposition — x = hi + lo (hi = bf16(x),
lo = bf16(x - hi)), x.y ~= hi.hi + hi.lo + lo.hi accumulated in fp32
PSUM — which runs at full bf16 PE rate, ~2.6x faster than fp32, with
~2^-18 per-product error (measured end-to-end rel err ~7e-6, same as
fp32).  The hi/lo splits of the kernel inputs are free: the host
passes them as bf16 pairs (same total bytes as fp32).

Row norms are never applied to the projection: the similarity matmul
uses the raw projection and the scalar engine computes
exp(raw * 1/(T*norm_row)) with a per-partition scale, which leaves
argmax and softmax unchanged.

Layouts: the PE contracts along the partition axis, so the host passes
features.T [D, B], W.T [D, E] and the prototype shard transposed
[E, K/8] (hi/lo bf16 pairs).  Host-side transposes/splits are pure
data movement (no FLOPs).
"""

import numpy as np
import ml_dtypes

import concourse.bass as bass
import concourse.bacc as bacc
import concourse.tile as tile
import concourse.mybir as mybir
from concourse import bass_utils

B = 1024          # batch
D = 1024          # feature dim (projection contraction)
E = 1024          # projection output dim (similarity contraction)
K = 32768         # prototypes
N_CORES = 8
KS = K // N_CORES  # 4096 prototypes per core
TEMP = 0.1

P = 128           # partitions
NBLK = 512        # matmul moving width / PSUM bank (fp32 out)
NB = B // P       # 8 batch tiles
NE = E // P       # 8 e tiles
ND = D // P       # 8 d tiles
NJ = KS // NBLK   # 8 k blocks per core
JC = 2            # k blocks per prototype DMA chunk
GB = 4            # batch tiles per group (SBUF budget for exp tiles)
NG = NB // GB     # 2 groups

F32 = mybir.dt.float32
BF16 = mybir.dt.bfloat16
U32 = mybir.dt.uint32

_CACHE = {}


def _build():
    if "nc" in _CACHE:
        return _CACHE["nc"]

    nc = bacc.Bacc("TRN2", target_bir_lowering=False, debug=False,
                   num_devices=N_CORES)

    featT_hi = nc.dram_tensor("featT_hi", [D, B], BF16, kind="ExternalInput").ap()
    featT_lo = nc.dram_tensor("featT_lo", [D, B], BF16, kind="ExternalInput").ap()
    WT_hi = nc.dram_tensor("WT_hi", [D, E], BF16, kind="ExternalInput").ap()
    WT_lo = nc.dram_tensor("WT_lo", [D, E], BF16, kind="ExternalInput").ap()
    bvec = nc.dram_tensor("bvec", [E], F32, kind="ExternalInput").ap()
    protT_hi = nc.dram_tensor("protT_hi", [E, KS], BF16, kind="ExternalInput").ap()
    protT_lo = nc.dram_tensor("protT_lo", [E, KS], BF16, kind="ExternalInput").ap()

    act = nc.dram_tensor("act", [B, KS], F32, kind="ExternalOutput").ap()
    lmax = nc.dram_tensor("lmax", [B, 8], F32, kind="ExternalOutput").ap()
    lidx = nc.dram_tensor("lidx", [B, 8], U32, kind="ExternalOutput").ap()

    AF = mybir.ActivationFunctionType
    ALU = mybir.AluOpType
    AX = mybir.AxisListType

    with tile.TileContext(nc) as tc:
        with tc.tile_pool(name="persist", bufs=1) as pp, \
             tc.tile_pool(name="psum", bufs=4, space="PSUM") as psum, \
             tc.tile_pool(name="psum_n", bufs=2, space="PSUM") as psum_n, \
             tc.tile_pool(name="dram", bufs=2, space="DRAM") as dram:

            btile = pp.tile([P, NE], F32, tag="btile", name="btile")
            nc.sync.dma_start(btile[:], bvec.rearrange("(t p) -> p t", p=P))
            ones = pp.tile([P, 1], F32, tag="ones", name="ones")
            nc.vector.memset(ones[:], 1.0)
            s_tile = pp.tile([P, NB], F32, tag="s_tile", name="s_tile")
            # bf16 hi/lo of the (unnormalized) projection, [e, b] layout
            pj_hi = [pp.tile([P, B], BF16, tag=f"pjh{e}", name=f"pjh{e}")
                     for e in range(NE)]
            pj_lo = [pp.tile([P, B], BF16, tag=f"pjl{e}", name=f"pjl{e}")
                     for e in range(NE)]

            # ---- phase 1: projection projT[e, b] = (features @ W.T + b).T
            with tc.tile_pool(name="inw", bufs=1) as p_in, \
                 tc.tile_pool(name="sq", bufs=2) as p_sq:
                fh, fl, wh, wl = [], [], [], []
                for d in range(ND):
                    t = p_in.tile([P, B], BF16, tag=f"fh{d}", name=f"fh{d}")
                    nc.sync.dma_start(t[:], featT_hi[d * P:(d + 1) * P, :])
                    fh.append(t)
                    t = p_in.tile([P, B], BF16, tag=f"fl{d}", name=f"fl{d}")
                    nc.sync.dma_start(t[:], featT_lo[d * P:(d + 1) * P, :])
                    fl.append(t)
                    t = p_in.tile([P, E], BF16, tag=f"wh{d}", name=f"wh{d}")
                    nc.sync.dma_start(t[:], WT_hi[d * P:(d + 1) * P, :])
                    wh.append(t)
                    t = p_in.tile([P, E], BF16, tag=f"wl{d}", name=f"wl{d}")
                    nc.sync.dma_start(t[:], WT_lo[d * P:(d + 1) * P, :])
                    wl.append(t)

                projT_sb = []
                for e in range(NE):
                    pj = p_in.tile([P, B], F32, tag=f"projT{e}",
                                   name=f"projT{e}")
                    projT_sb.append(pj)
                    es = slice(e * P, (e + 1) * P)
                    for h in range(B // NBLK):
                        hs = slice(h * NBLK, (h + 1) * NBLK)
                        ps = psum.tile([P, NBLK], F32, tag="ps", name="ps")
                        for d in range(ND):
                            nc.tensor.matmul(ps[:], lhsT=wh[d][:, es],
                                             rhs=fh[d][:, hs],
                                             start=(d == 0), stop=False)
                            nc.tensor.matmul(ps[:], lhsT=wh[d][:, es],
                                             rhs=fl[d][:, hs],
                                             start=False, stop=False)
                            nc.tensor.matmul(ps[:], lhsT=wl[d][:, es],
                                             rhs=fh[d][:, hs],
                                             start=False, stop=(d == ND - 1))
                        nc.vector.tensor_scalar_add(pj[:, hs], ps[:],
                                                    btile[:, e:e + 1])
                    # bf16 hi/lo split of this projection tile
                    nc.vector.tensor_copy(pj_hi[e][:], pj[:])
                    nc.vector.tensor_sub(pj_lo[e][:], pj[:], pj_hi[e][:])

                # ---- phase 2: row norms -> s_row = 1/(T*norm_row)
                # ssum[p, b] = sum over e-tiles of projT^2; then one tiny
                # fp32 matmul per b-tile reduces over the partition axis.
                ssum = p_sq.tile([P, B], F32, tag="ssum", name="ssum", bufs=1)
                sq = p_sq.tile([P, B], F32, tag="sqt", name="sqt", bufs=1)
                nc.vector.tensor_mul(ssum[:], projT_sb[0][:], projT_sb[0][:])
                for e in range(1, NE):
                    nc.vector.tensor_mul(sq[:], projT_sb[e][:], projT_sb[e][:])
                    nc.vector.tensor_add(ssum[:], ssum[:], sq[:])
                for bt in range(NB):
                    pnb = psum_n.tile([P, 1], F32, tag="pnb", name="pnb")
                    nc.tensor.matmul(pnb[:],
                                     lhsT=ssum[:, bt * P:(bt + 1) * P],
                                     rhs=ones[:], start=True, stop=True)
                    rec = p_sq.tile([P, 1], F32, tag="rec", name="rec")
                    nc.vector.reciprocal(rec[:], pnb[:])
                    # sqrt((1/norm^2) * (1/T^2)) = 1/(T*norm)
                    nc.scalar.activation(s_tile[:, bt:bt + 1], rec[:], AF.Sqrt,
                                         scale=1.0 / (TEMP * TEMP))

            # ---- phase 3: similarities, exp, denominators, argmax
            with tc.tile_pool(name="sims", bufs=5) as p_sims, \
                 tc.tile_pool(name="pt", bufs=2) as p_pt, \
                 tc.tile_pool(name="small", bufs=2) as p_sm:
                for g in range(NG):
                    exp_t = [p_sims.tile([P, KS], F32, tag="exp",
                                         name=f"exp_g{g}_{bl}")
                             for bl in range(GB)]
                    dsum_t = [p_sm.tile([P, NJ], F32, tag=f"dsum{bl}",
                                        name=f"dsum{bl}")
                              for bl in range(GB)]
                    for jc in range(NJ // JC):
                        pth, ptl = [], []
                        cs = slice(jc * JC * NBLK, (jc + 1) * JC * NBLK)
                        for e in range(NE):
                            t = p_pt.tile([P, JC * NBLK], BF16, tag=f"pth{e}",
                                          name=f"pth{e}")
                            nc.sync.dma_start(t[:], protT_hi[e * P:(e + 1) * P, cs])
                            pth.append(t)
                            t = p_pt.tile([P, JC * NBLK], BF16, tag=f"ptl{e}",
                                          name=f"ptl{e}")
                            nc.sync.dma_start(t[:], protT_lo[e * P:(e + 1) * P, cs])
                            ptl.append(t)
                        for ji in range(JC):
                            j = jc * JC + ji
                            js = slice(ji * NBLK, (ji + 1) * NBLK)
                            for bl in range(GB):
                                bt = g * GB + bl
                                bs = slice(bt * P, (bt + 1) * P)
                                ps = psum.tile([P, NBLK], F32, tag="ps",
                                               name="ps")
                                for e in range(NE):
                                    nc.tensor.matmul(ps[:],
                                                     lhsT=pj_hi[e][:, bs],
                                                     rhs=pth[e][:, js],
                                                     start=(e == 0), stop=False)
                                    nc.tensor.matmul(ps[:],
                                                     lhsT=pj_hi[e][:, bs],
                                                     rhs=ptl[e][:, js],
                                                     start=False, stop=False)
                                    nc.tensor.matmul(ps[:],
                                                     lhsT=pj_lo[e][:, bs],
                                                     rhs=pth[e][:, js],
                                                     start=False,
                                                     stop=(e == NE - 1))
                                nc.scalar.activation(
                                    exp_t[bl][:, j * NBLK:(j + 1) * NBLK],
                                    ps[:], AF.Exp,
                                    scale=s_tile[:, bt:bt + 1],
                                    accum_out=dsum_t[bl][:, j:j + 1])

                    # local argmax on the unnormalized numerators (argmax is
                    # invariant to the positive per-row normalization, and
                    # the values are cross-core comparable since every core
                    # uses identical row scales) — runs before the collective.
                    for bl in range(GB):
                        bt = g * GB + bl
                        m8 = p_sm.tile([P, 8], F32, tag="m8", name="m8")
                        i8 = p_sm.tile([P, 8], U32, tag="i8", name="i8")
                        nc.vector.max(m8[:], exp_t[bl][:])
                        nc.vector.max_index(i8[:], m8[:], exp_t[bl][:])
                        nc.sync.dma_start(lmax[bt * P:(bt + 1) * P, :], m8[:])
                        nc.sync.dma_start(lidx[bt * P:(bt + 1) * P, :], i8[:])

                    den = p_sm.tile([P, GB], F32, tag="den", name="den")
                    for bl in range(GB):
                        nc.vector.tensor_reduce(den[:, bl:bl + 1], dsum_t[bl][:],
                                                axis=AX.X, op=ALU.add)
                    cin = dram.tile([P, GB], F32, tag="cin", name="cin")
                    cout = dram.tile([P, GB], F32, tag="cout", name="cout")
                    nc.sync.dma_start(cin[:], den[:])
                    nc.gpsimd.collective_compute(
                        "AllReduce", ALU.add,
                        replica_groups=[list(range(N_CORES))],
                        ins=[cin.opt()], outs=[cout.opt()])
                    gd = p_sm.tile([P, GB], F32, tag="gd", name="gd")
                    nc.sync.dma_start(gd[:], cout[:])
                    rd = p_sm.tile([P, GB], F32, tag="rd", name="rd")
                    nc.vector.reciprocal(rd[:], gd[:])

                    for bl in range(GB):
                        bt = g * GB + bl
                        nc.vector.tensor_scalar_mul(exp_t[bl][:], exp_t[bl][:],
                                                    rd[:, bl:bl + 1])
                        nc.sync.dma_start(act[bt * P:(bt + 1) * P, :],
                                          exp_t[bl][:])

    nc.compile()
    _CACHE["nc"] = nc
    return nc


def _split_bf16(x):
    hi = x.astype(ml_dtypes.bfloat16)
    lo = (x - hi.astype(np.float32)).astype(ml_dtypes.bfloat16)
    return np.ascontiguousarray(hi), np.ascontiguousarray(lo)


def kernel(features, W, b, prototypes, _run_kwargs=None):
    nc = _build()

    featT = np.asarray(features, dtype=np.float32).T
    WT = np.asarray(W, dtype=np.float32).T
    bvec = np.asarray(b, dtype=np.float32)
    prototypes = np.asarray(prototypes, dtype=np.float32)

    fh, fl = _split_bf16(featT)
    wh, wl = _split_bf16(WT)

    in_maps = []
    for c in range(N_CORES):
        ph, pl = _split_bf16(prototypes[c * KS:(c + 1) * KS].T)
        in_maps.append({"featT_hi": fh, "featT_lo": fl,
                        "WT_hi": wh, "WT_lo": wl, "bvec": bvec,
                        "protT_hi": ph, "protT_lo": pl})

    res = bass_utils.run_bass_kernel_spmd(
        nc, in_maps, core_ids=list(range(N_CORES)), **(_run_kwargs or {}))
    if _run_kwargs:
        _CACHE["last_result"] = res

    act = np.concatenate([res.results[c]["act"] for c in range(N_CORES)],
                         axis=1)
    lmax = np.stack([res.results[c]["lmax"][:, 0] for c in range(N_CORES)])
    lidx = np.stack([res.results[c]["lidx"][:, 0] for c in range(N_CORES)])
    best_core = np.argmax(lmax, axis=0)                       # [B]
    rows = np.arange(B)
    best_idx = (best_core * KS + lidx[best_core, rows]).astype(np.int32)
    return act, best_idx


# revision 8
# speedup vs baseline: 1.3620x; 1.0500x over previous
"""VQ codebook kernel for 8 TRN2 NeuronCores.

Computation (matches the reference):
    projected = features @ W.T + b            [B, D]
    projected = l2_normalize(projected)        [B, D]
    sims      = projected @ prototypes.T       [B, K]
    act       = softmax(sims / T, axis=-1)     [B, K]
    best_idx  = argmax(sims, axis=-1)          [B]

Distribution: prototypes are sharded on the K axis across the 8 cores
(tensor-parallel codebook).  Every core computes the full projection
(cheap relative to the similarity matmul), its [B, K/8] similarity
block, exp() with the row 1/(T*norm) scale folded in, a local softmax
numerator + row-sum, then the row-sums are AllReduce'd across cores and
each core normalizes + writes its activation block.  Local (max, argmax)
per row are written out and combined on the host (a 16 KB gather).

Precision/speed: fp32 matmuls on the PE cost 4 cycles/row (two
half-speed passes + per-pass weight reloads).  Instead every matmul
runs as a 3-pass bf16 decomposition — x = hi + lo (hi = bf16(x),
lo = bf16(x - hi)), x.y ~= hi.hi + hi.lo + lo.hi accumulated in fp32
PSUM — which runs at full bf16 PE rate, ~2.6x faster than fp32, with
~2^-18 per-product error (measured end-to-end rel err ~7e-6, same as
fp32).  The hi/lo splits of the kernel inputs are free: the host
passes them as bf16 pairs (same total bytes as fp32).

Row norms are never applied to the projection: the similarity matmul
uses the raw projection and the scalar engine computes
exp(raw * 1/(T*norm_row)) with a per-partition scale, which leaves
argmax and softmax unchanged.

Layouts: the PE contracts along the partition axis, so the host passes
features.T [D, B], W.T [D, E] and the prototype shard transposed
[E, K/8] (hi/lo bf16 pairs).  Host-side transposes/splits are pure
data movement (no FLOPs).
"""

import numpy as np
import ml_dtypes

import concourse.bass as bass
import concourse.bacc as bacc
import concourse.tile as tile
import concourse.mybir as mybir
from concourse import bass_utils

B = 1024          # batch
D = 1024          # feature dim (projection contraction)
E = 1024          # projection output dim (similarity contraction)
K = 32768         # prototypes
N_CORES = 8
KS = K // N_CORES  # 4096 prototypes per core
TEMP = 0.1

P = 128           # partitions
NBLK = 512        # matmul moving width / PSUM bank (fp32 out)
NB = B // P       # 8 batch tiles
NE = E // P       # 8 e tiles
ND = D // P       # 8 d tiles
NJ = KS // NBLK   # 8 k blocks per core
JC = 1            # k blocks per prototype DMA chunk
GB = 4            # batch tiles per group (SBUF budget for exp tiles)
NG = NB // GB     # 2 groups

F32 = mybir.dt.float32
BF16 = mybir.dt.bfloat16
U32 = mybir.dt.uint32

_CACHE = {}


def _build():
    if "nc" in _CACHE:
        return _CACHE["nc"]

    nc = bacc.Bacc("TRN2", target_bir_lowering=False, debug=False,
                   num_devices=N_CORES)

    featT_hi = nc.dram_tensor("featT_hi", [D, B], BF16, kind="ExternalInput").ap()
    featT_lo = nc.dram_tensor("featT_lo", [D, B], BF16, kind="ExternalInput").ap()
    WT_hi = nc.dram_tensor("WT_hi", [D, E], BF16, kind="ExternalInput").ap()
    WT_lo = nc.dram_tensor("WT_lo", [D, E], BF16, kind="ExternalInput").ap()
    bvec = nc.dram_tensor("bvec", [E], F32, kind="ExternalInput").ap()
    protT_hi = nc.dram_tensor("protT_hi", [E, KS], BF16, kind="ExternalInput").ap()
    protT_lo = nc.dram_tensor("protT_lo", [E, KS], BF16, kind="ExternalInput").ap()

    act = nc.dram_tensor("act", [B, KS], F32, kind="ExternalOutput").ap()
    lmax = nc.dram_tensor("lmax", [B, 8], F32, kind="ExternalOutput").ap()
    lidx = nc.dram_tensor("lidx", [B, 8], U32, kind="ExternalOutput").ap()

    AF = mybir.ActivationFunctionType
    ALU = mybir.AluOpType
    AX = mybir.AxisListType

    with tile.TileContext(nc) as tc:
        with tc.tile_pool(name="persist", bufs=1) as pp, \
             tc.tile_pool(name="psum", bufs=4, space="PSUM") as psum, \
             tc.tile_pool(name="psum_n", bufs=2, space="PSUM") as psum_n, \
             tc.tile_pool(name="dram", bufs=2, space="DRAM") as dram:

            btile = pp.tile([P, NE], F32, tag="btile", name="btile")
            nc.sync.dma_start(btile[:], bvec.rearrange("(t p) -> p t", p=P))
            ones = pp.tile([P, 1], F32, tag="ones", name="ones")
            nc.vector.memset(ones[:], 1.0)
            s_tile = pp.tile([P, NB], F32, tag="s_tile", name="s_tile")
            # bf16 hi/lo of the (unnormalized) projection, [e, b] layout
            pj_hi = [pp.tile([P, B], BF16, tag=f"pjh{e}", name=f"pjh{e}")
                     for e in range(NE)]
            pj_lo = [pp.tile([P, B], BF16, tag=f"pjl{e}", name=f"pjl{e}")
                     for e in range(NE)]

            # ---- phase 1: projection projT[e, b] = (features @ W.T + b).T
            with tc.tile_pool(name="inw", bufs=1) as p_in, \
                 tc.tile_pool(name="sq", bufs=2) as p_sq:
                fh, fl, wh, wl = [], [], [], []
                for d in range(ND):
                    t = p_in.tile([P, B], BF16, tag=f"fh{d}", name=f"fh{d}")
                    nc.sync.dma_start(t[:], featT_hi[d * P:(d + 1) * P, :])
                    fh.append(t)
                    t = p_in.tile([P, B], BF16, tag=f"fl{d}", name=f"fl{d}")
                    nc.sync.dma_start(t[:], featT_lo[d * P:(d + 1) * P, :])
                    fl.append(t)
                    t = p_in.tile([P, E], BF16, tag=f"wh{d}", name=f"wh{d}")
                    nc.sync.dma_start(t[:], WT_hi[d * P:(d + 1) * P, :])
                    wh.append(t)
                    t = p_in.tile([P, E], BF16, tag=f"wl{d}", name=f"wl{d}")
                    nc.sync.dma_start(t[:], WT_lo[d * P:(d + 1) * P, :])
                    wl.append(t)

                projT_sb = []
                for e in range(NE):
                    pj = p_in.tile([P, B], F32, tag=f"projT{e}",
                                   name=f"projT{e}")
                    projT_sb.append(pj)
                    es = slice(e * P, (e + 1) * P)
                    for h in range(B // NBLK):
                        hs = slice(h * NBLK, (h + 1) * NBLK)
                        ps = psum.tile([P, NBLK], F32, tag="ps", name="ps")
                        for d in range(ND):
                            nc.tensor.matmul(ps[:], lhsT=wh[d][:, es],
                                             rhs=fh[d][:, hs],
                                             start=(d == 0), stop=False)
                            nc.tensor.matmul(ps[:], lhsT=wh[d][:, es],
                                             rhs=fl[d][:, hs],
                                             start=False, stop=False)
                            nc.tensor.matmul(ps[:], lhsT=wl[d][:, es],
                                             rhs=fh[d][:, hs],
                                             start=False, stop=(d == ND - 1))
                        nc.vector.tensor_scalar_add(pj[:, hs], ps[:],
                                                    btile[:, e:e + 1])
                    # bf16 hi/lo split of this projection tile; hi-round on
                    # the scalar engine and subtract on gpsimd (both idle
                    # here) so the vector engine isn't the critical path
                    # into the similarity phase.
                    nc.scalar.copy(pj_hi[e][:], pj[:])
                    nc.gpsimd.tensor_sub(pj_lo[e][:], pj[:], pj_hi[e][:])

                # ---- phase 2: row norms -> s_row = 1/(T*norm_row)
                # ssum[p, b] = sum over e-tiles of projT^2; then one tiny
                # fp32 matmul per b-tile reduces over the partition axis.
                ssum = p_sq.tile([P, B], F32, tag="ssum", name="ssum", bufs=1)
                sq = p_sq.tile([P, B], F32, tag="sqt", name="sqt", bufs=1)
                nc.vector.tensor_mul(ssum[:], projT_sb[0][:], projT_sb[0][:])
                for e in range(1, NE):
                    nc.vector.tensor_mul(sq[:], projT_sb[e][:], projT_sb[e][:])
                    nc.vector.tensor_add(ssum[:], ssum[:], sq[:])
                for bt in range(NB):
                    pnb = psum_n.tile([P, 1], F32, tag="pnb", name="pnb")
                    nc.tensor.matmul(pnb[:],
                                     lhsT=ssum[:, bt * P:(bt + 1) * P],
                                     rhs=ones[:], start=True, stop=True)
                    rec = p_sq.tile([P, 1], F32, tag="rec", name="rec")
                    nc.vector.reciprocal(rec[:], pnb[:])
                    # sqrt((1/norm^2) * (1/T^2)) = 1/(T*norm)
                    nc.scalar.activation(s_tile[:, bt:bt + 1], rec[:], AF.Sqrt,
                                         scale=1.0 / (TEMP * TEMP))

            # ---- phase 3: similarities, exp, denominators, argmax
            with tc.tile_pool(name="sims", bufs=7) as p_sims, \
                 tc.tile_pool(name="pt", bufs=2) as p_pt, \
                 tc.tile_pool(name="small", bufs=2) as p_sm:
                for g in range(NG):
                    exp_t = [p_sims.tile([P, KS], F32, tag="exp",
                                         name=f"exp_g{g}_{bl}")
                             for bl in range(GB)]
                    dsum_t = [p_sm.tile([P, NJ], F32, tag=f"dsum{bl}",
                                        name=f"dsum{bl}")
                              for bl in range(GB)]
                    for jc in range(NJ // JC):
                        pth, ptl = [], []
                        cs = slice(jc * JC * NBLK, (jc + 1) * JC * NBLK)
                        for e in range(NE):
                            t = p_pt.tile([P, JC * NBLK], BF16, tag=f"pth{e}",
                                          name=f"pth{e}")
                            nc.sync.dma_start(t[:], protT_hi[e * P:(e + 1) * P, cs])
                            pth.append(t)
                            t = p_pt.tile([P, JC * NBLK], BF16, tag=f"ptl{e}",
                                          name=f"ptl{e}")
                            nc.sync.dma_start(t[:], protT_lo[e * P:(e + 1) * P, cs])
                            ptl.append(t)
                        for ji in range(JC):
                            j = jc * JC + ji
                            js = slice(ji * NBLK, (ji + 1) * NBLK)
                            for bl in range(GB):
                                bt = g * GB + bl
                                bs = slice(bt * P, (bt + 1) * P)
                                ps = psum.tile([P, NBLK], F32, tag="ps",
                                               name="ps")
                                for e in range(NE):
                                    nc.tensor.matmul(ps[:],
                                                     lhsT=pj_hi[e][:, bs],
                                                     rhs=pth[e][:, js],
                                                     start=(e == 0), stop=False)
                                    nc.tensor.matmul(ps[:],
                                                     lhsT=pj_hi[e][:, bs],
                                                     rhs=ptl[e][:, js],
                                                     start=False, stop=False)
                                    nc.tensor.matmul(ps[:],
                                                     lhsT=pj_lo[e][:, bs],
                                                     rhs=pth[e][:, js],
                                                     start=False,
                                                     stop=(e == NE - 1))
                                nc.scalar.activation(
                                    exp_t[bl][:, j * NBLK:(j + 1) * NBLK],
                                    ps[:], AF.Exp,
                                    scale=s_tile[:, bt:bt + 1],
                                    accum_out=dsum_t[bl][:, j:j + 1])

                    # local argmax on the unnormalized numerators (argmax is
                    # invariant to the positive per-row normalization, and
                    # the values are cross-core comparable since every core
                    # uses identical row scales) — runs before the collective.
                    for bl in range(GB):
                        bt = g * GB + bl
                        m8 = p_sm.tile([P, 8], F32, tag="m8", name="m8")
                        i8 = p_sm.tile([P, 8], U32, tag="i8", name="i8")
                        nc.vector.max(m8[:], exp_t[bl][:])
                        nc.vector.max_index(i8[:], m8[:], exp_t[bl][:])
                        nc.sync.dma_start(lmax[bt * P:(bt + 1) * P, :], m8[:])
                        nc.sync.dma_start(lidx[bt * P:(bt + 1) * P, :], i8[:])

                    den = p_sm.tile([P, GB], F32, tag="den", name="den")
                    for bl in range(GB):
                        nc.vector.tensor_reduce(den[:, bl:bl + 1], dsum_t[bl][:],
                                                axis=AX.X, op=ALU.add)
                    cin = dram.tile([P, GB], F32, tag="cin", name="cin")
                    cout = dram.tile([P, GB], F32, tag="cout", name="cout")
                    nc.sync.dma_start(cin[:], den[:])
                    nc.gpsimd.collective_compute(
                        "AllReduce", ALU.add,
                        replica_groups=[list(range(N_CORES))],
                        ins=[cin.opt()], outs=[cout.opt()])
                    gd = p_sm.tile([P, GB], F32, tag="gd", name="gd")
                    nc.sync.dma_start(gd[:], cout[:])
                    rd = p_sm.tile([P, GB], F32, tag="rd", name="rd")
                    nc.vector.reciprocal(rd[:], gd[:])

                    for bl in range(GB):
                        bt = g * GB + bl
                        nc.vector.tensor_scalar_mul(exp_t[bl][:], exp_t[bl][:],
                                                    rd[:, bl:bl + 1])
                        nc.sync.dma_start(act[bt * P:(bt + 1) * P, :],
                                          exp_t[bl][:])

    nc.compile()
    _CACHE["nc"] = nc
    return nc


def _split_bf16(x):
    hi = x.astype(ml_dtypes.bfloat16)
    lo = (x - hi.astype(np.float32)).astype(ml_dtypes.bfloat16)
    return np.ascontiguousarray(hi), np.ascontiguousarray(lo)


def kernel(features, W, b, prototypes, _run_kwargs=None):
    nc = _build()

    featT = np.asarray(features, dtype=np.float32).T
    WT = np.asarray(W, dtype=np.float32).T
    bvec = np.asarray(b, dtype=np.float32)
    prototypes = np.asarray(prototypes, dtype=np.float32)

    fh, fl = _split_bf16(featT)
    wh, wl = _split_bf16(WT)

    in_maps = []
    for c in range(N_CORES):
        ph, pl = _split_bf16(prototypes[c * KS:(c + 1) * KS].T)
        in_maps.append({"featT_hi": fh, "featT_lo": fl,
                        "WT_hi": wh, "WT_lo": wl, "bvec": bvec,
                        "protT_hi": ph, "protT_lo": pl})

    res = bass_utils.run_bass_kernel_spmd(
        nc, in_maps, core_ids=list(range(N_CORES)), **(_run_kwargs or {}))
    if _run_kwargs:
        _CACHE["last_result"] = res

    act = np.concatenate([res.results[c]["act"] for c in range(N_CORES)],
                         axis=1)
    lmax = np.stack([res.results[c]["lmax"][:, 0] for c in range(N_CORES)])
    lidx = np.stack([res.results[c]["lidx"][:, 0] for c in range(N_CORES)])
    best_core = np.argmax(lmax, axis=0)                       # [B]
    rows = np.arange(B)
    best_idx = (best_core * KS + lidx[best_core, rows]).astype(np.int32)
    return act, best_idx


# revision 12
# speedup vs baseline: 1.3808x; 1.0138x over previous
"""VQ codebook kernel for 8 TRN2 NeuronCores.

Computation (matches the reference):
    projected = features @ W.T + b            [B, D]
    projected = l2_normalize(projected)        [B, D]
    sims      = projected @ prototypes.T       [B, K]
    act       = softmax(sims / T, axis=-1)     [B, K]
    best_idx  = argmax(sims, axis=-1)          [B]

Distribution: prototypes are sharded on the K axis across the 8 cores
(tensor-parallel codebook).  Every core computes the full projection
(cheap relative to the similarity matmul), its [B, K/8] similarity
block, exp() with the row 1/(T*norm) scale folded in, a local softmax
numerator + row-sum, then the row-sums are AllReduce'd across cores and
each core normalizes + writes its activation block.  Local (max, argmax)
per row are written out and combined on the host (a 16 KB gather).

Precision/speed: fp32 matmuls on the PE cost 4 cycles/row (two
half-speed passes + per-pass weight reloads).  Instead every matmul
runs as a 3-pass bf16 decomposition — x = hi + lo (hi = bf16(x),
lo = bf16(x - hi)), x.y ~= hi.hi + hi.lo + lo.hi accumulated in fp32
PSUM — which runs at full bf16 PE rate, ~2.6x faster than fp32, with
~2^-18 per-product error (measured end-to-end rel err ~7e-6, same as
fp32).  The hi/lo splits of the kernel inputs are free: the host
passes them as bf16 pairs (same total bytes as fp32).

Row norms are never applied to the projection: the similarity matmul
uses the raw projection and the scalar engine computes
exp(raw * 1/(T*norm_row)) with a per-partition scale, which leaves
argmax and softmax unchanged.

Layouts: the PE contracts along the partition axis, so the host passes
features.T [D, B], W.T [D, E] and the prototype shard transposed
[E, K/8] (hi/lo bf16 pairs).  Host-side transposes/splits are pure
data movement (no FLOPs).
"""

import numpy as np
import ml_dtypes

import concourse.bass as bass
import concourse.bacc as bacc
import concourse.tile as tile
import concourse.mybir as mybir
from concourse import bass_utils

B = 1024          # batch
D = 1024          # feature dim (projection contraction)
E = 1024          # projection output dim (similarity contraction)
K = 32768         # prototypes
N_CORES = 8
KS = K // N_CORES  # 4096 prototypes per core
TEMP = 0.1

P = 128           # partitions
NBLK = 512        # matmul moving width / PSUM bank (fp32 out)
NB = B // P       # 8 batch tiles
NE = E // P       # 8 e tiles
ND = D // P       # 8 d tiles
NJ = KS // NBLK   # 8 k blocks per core
JC = 1            # k blocks per prototype DMA chunk
GB = 4            # batch tiles per group (SBUF budget for exp tiles)
NG = NB // GB     # 2 groups

F32 = mybir.dt.float32
BF16 = mybir.dt.bfloat16
U32 = mybir.dt.uint32

_CACHE = {}


def _build():
    if "nc" in _CACHE:
        return _CACHE["nc"]

    nc = bacc.Bacc("TRN2", target_bir_lowering=False, debug=False,
                   num_devices=N_CORES)

    featT_hi = nc.dram_tensor("featT_hi", [D, B], BF16, kind="ExternalInput").ap()
    featT_lo = nc.dram_tensor("featT_lo", [D, B], BF16, kind="ExternalInput").ap()
    WT_hi = nc.dram_tensor("WT_hi", [D, E], BF16, kind="ExternalInput").ap()
    WT_lo = nc.dram_tensor("WT_lo", [D, E], BF16, kind="ExternalInput").ap()
    bvec = nc.dram_tensor("bvec", [E], F32, kind="ExternalInput").ap()
    protT_hi = nc.dram_tensor("protT_hi", [E, KS], BF16, kind="ExternalInput").ap()
    protT_lo = nc.dram_tensor("protT_lo", [E, KS], BF16, kind="ExternalInput").ap()

    act = nc.dram_tensor("act", [B, KS], F32, kind="ExternalOutput").ap()
    lmax = nc.dram_tensor("lmax", [B, 8], F32, kind="ExternalOutput").ap()
    lidx = nc.dram_tensor("lidx", [B, 8], U32, kind="ExternalOutput").ap()

    AF = mybir.ActivationFunctionType
    ALU = mybir.AluOpType
    AX = mybir.AxisListType

    with tile.TileContext(nc) as tc:
        with tc.tile_pool(name="persist", bufs=1) as pp, \
             tc.tile_pool(name="psum", bufs=4, space="PSUM") as psum, \
             tc.tile_pool(name="psum_n", bufs=2, space="PSUM") as psum_n, \
             tc.tile_pool(name="dram", bufs=2, space="DRAM") as dram:

            btile = pp.tile([P, NE], F32, tag="btile", name="btile")
            nc.sync.dma_start(btile[:], bvec.rearrange("(t p) -> p t", p=P))
            ones = pp.tile([P, 1], F32, tag="ones", name="ones")
            nc.vector.memset(ones[:], 1.0)
            s_tile = pp.tile([P, NB], F32, tag="s_tile", name="s_tile")
            # bf16 hi/lo of the (unnormalized) projection, [e, b] layout
            pj_hi = [pp.tile([P, B], BF16, tag=f"pjh{e}", name=f"pjh{e}")
                     for e in range(NE)]
            pj_lo = [pp.tile([P, B], BF16, tag=f"pjl{e}", name=f"pjl{e}")
                     for e in range(NE)]

            # ---- phase 1: projection projT[e, b] = (features @ W.T + b).T
            with tc.tile_pool(name="inw", bufs=1) as p_in, \
                 tc.tile_pool(name="sq", bufs=2) as p_sq:
                fh, fl, wh, wl = [], [], [], []
                for d in range(ND):
                    t = p_in.tile([P, B], BF16, tag=f"fh{d}", name=f"fh{d}")
                    nc.sync.dma_start(t[:], featT_hi[d * P:(d + 1) * P, :])
                    fh.append(t)
                    t = p_in.tile([P, B], BF16, tag=f"fl{d}", name=f"fl{d}")
                    nc.sync.dma_start(t[:], featT_lo[d * P:(d + 1) * P, :])
                    fl.append(t)
                    t = p_in.tile([P, E], BF16, tag=f"wh{d}", name=f"wh{d}")
                    nc.sync.dma_start(t[:], WT_hi[d * P:(d + 1) * P, :])
                    wh.append(t)
                    t = p_in.tile([P, E], BF16, tag=f"wl{d}", name=f"wl{d}")
                    nc.sync.dma_start(t[:], WT_lo[d * P:(d + 1) * P, :])
                    wl.append(t)

                projT_sb = []
                for e in range(NE):
                    pj = p_in.tile([P, B], F32, tag=f"projT{e}",
                                   name=f"projT{e}")
                    projT_sb.append(pj)
                    es = slice(e * P, (e + 1) * P)
                    for h in range(B // NBLK):
                        hs = slice(h * NBLK, (h + 1) * NBLK)
                        ps = psum.tile([P, NBLK], F32, tag="ps", name="ps")
                        for d in range(ND):
                            nc.tensor.matmul(ps[:], lhsT=wh[d][:, es],
                                             rhs=fh[d][:, hs],
                                             start=(d == 0), stop=False)
                            nc.tensor.matmul(ps[:], lhsT=wh[d][:, es],
                                             rhs=fl[d][:, hs],
                                             start=False, stop=False)
                            nc.tensor.matmul(ps[:], lhsT=wl[d][:, es],
                                             rhs=fh[d][:, hs],
                                             start=False, stop=(d == ND - 1))
                        nc.vector.tensor_scalar_add(pj[:, hs], ps[:],
                                                    btile[:, e:e + 1])
                    # bf16 hi/lo split of this projection tile; hi-round on
                    # the scalar engine and subtract on gpsimd (both idle
                    # here) so the vector engine isn't the critical path
                    # into the similarity phase.
                    nc.scalar.copy(pj_hi[e][:], pj[:])
                    nc.gpsimd.tensor_sub(pj_lo[e][:], pj[:], pj_hi[e][:])

                # ---- phase 2: row norms -> s_row = 1/(T*norm_row)
                # ssum[p, b] = sum over e-tiles of projT^2; then one tiny
                # fp32 matmul per b-tile reduces over the partition axis.
                ssum = p_sq.tile([P, B], F32, tag="ssum", name="ssum", bufs=1)
                sq = p_sq.tile([P, B], F32, tag="sqt", name="sqt", bufs=1)
                nc.vector.tensor_mul(ssum[:], projT_sb[0][:], projT_sb[0][:])
                for e in range(1, NE):
                    nc.vector.tensor_mul(sq[:], projT_sb[e][:], projT_sb[e][:])
                    nc.vector.tensor_add(ssum[:], ssum[:], sq[:])
                for bt in range(NB):
                    pnb = psum_n.tile([P, 1], F32, tag="pnb", name="pnb")
                    nc.tensor.matmul(pnb[:],
                                     lhsT=ssum[:, bt * P:(bt + 1) * P],
                                     rhs=ones[:], start=True, stop=True)
                    rec = p_sq.tile([P, 1], F32, tag="rec", name="rec")
                    nc.vector.reciprocal(rec[:], pnb[:])
                    # sqrt((1/norm^2) * (1/T^2)) = 1/(T*norm)
                    nc.scalar.activation(s_tile[:, bt:bt + 1], rec[:], AF.Sqrt,
                                         scale=1.0 / (TEMP * TEMP))

            # ---- phase 3: similarities, exp, denominators, argmax
            with tc.tile_pool(name="sims", bufs=7) as p_sims, \
                 tc.tile_pool(name="pt", bufs=2) as p_pt, \
                 tc.tile_pool(name="small", bufs=2) as p_sm:
                for g in range(NG):
                    exp_t = [p_sims.tile([P, KS], F32, tag="exp",
                                         name=f"exp_g{g}_{bl}")
                             for bl in range(GB)]
                    dsum_t = [p_sm.tile([P, NJ], F32, tag=f"dsum{bl}",
                                        name=f"dsum{bl}")
                              for bl in range(GB)]
                    # per-j-block top-8 maxima, folded at group end — keeps
                    # the big MAX8 scans off the post-matmul critical path
                    mblk_t = [p_sm.tile([P, 8 * NJ], F32, tag=f"mblk{bl}",
                                        name=f"mblk{bl}")
                              for bl in range(GB)]
                    for jc in range(NJ // JC):
                        pth, ptl = [], []
                        cs = slice(jc * JC * NBLK, (jc + 1) * JC * NBLK)
                        for e in range(NE):
                            t = p_pt.tile([P, JC * NBLK], BF16, tag=f"pth{e}",
                                          name=f"pth{e}")
                            nc.sync.dma_start(t[:], protT_hi[e * P:(e + 1) * P, cs])
                            pth.append(t)
                            t = p_pt.tile([P, JC * NBLK], BF16, tag=f"ptl{e}",
                                          name=f"ptl{e}")
                            nc.sync.dma_start(t[:], protT_lo[e * P:(e + 1) * P, cs])
                            ptl.append(t)
                        for ji in range(JC):
                            j = jc * JC + ji
                            js = slice(ji * NBLK, (ji + 1) * NBLK)
                            for bl in range(GB):
                                bt = g * GB + bl
                                bs = slice(bt * P, (bt + 1) * P)
                                ps = psum.tile([P, NBLK], F32, tag="ps",
                                               name="ps")
                                for e in range(NE):
                                    nc.tensor.matmul(ps[:],
                                                     lhsT=pj_hi[e][:, bs],
                                                     rhs=pth[e][:, js],
                                                     start=(e == 0), stop=False)
                                    nc.tensor.matmul(ps[:],
                                                     lhsT=pj_hi[e][:, bs],
                                                     rhs=ptl[e][:, js],
                                                     start=False, stop=False)
                                    nc.tensor.matmul(ps[:],
                                                     lhsT=pj_lo[e][:, bs],
                                                     rhs=pth[e][:, js],
                                                     start=False,
                                                     stop=(e == NE - 1))
                                nc.scalar.activation(
                                    exp_t[bl][:, j * NBLK:(j + 1) * NBLK],
                                    ps[:], AF.Exp,
                                    scale=s_tile[:, bt:bt + 1],
                                    accum_out=dsum_t[bl][:, j:j + 1])
                                nc.vector.max(
                                    mblk_t[bl][:, j * 8:(j + 1) * 8],
                                    exp_t[bl][:, j * NBLK:(j + 1) * NBLK])

                    # local argmax on the unnormalized numerators (argmax is
                    # invariant to the positive per-row normalization, and
                    # the values are cross-core comparable since every core
                    # uses identical row scales) — runs before the collective.
                    for bl in range(GB):
                        bt = g * GB + bl
                        m8 = p_sm.tile([P, 8], F32, tag="m8", name="m8")
                        i8 = p_sm.tile([P, 8], U32, tag="i8", name="i8")
                        nc.vector.max(m8[:], mblk_t[bl][:])
                        nc.vector.max_index(i8[:], m8[:], exp_t[bl][:])
                        nc.sync.dma_start(lmax[bt * P:(bt + 1) * P, :], m8[:])
                        nc.sync.dma_start(lidx[bt * P:(bt + 1) * P, :], i8[:])

                    den = p_sm.tile([P, GB], F32, tag="den", name="den")
                    for bl in range(GB):
                        nc.vector.tensor_reduce(den[:, bl:bl + 1], dsum_t[bl][:],
                                                axis=AX.X, op=ALU.add)
                    cin = dram.tile([P, GB], F32, tag="cin", name="cin")
                    cout = dram.tile([P, GB], F32, tag="cout", name="cout")
                    nc.sync.dma_start(cin[:], den[:])
                    nc.gpsimd.collective_compute(
                        "AllReduce", ALU.add,
                        replica_groups=[list(range(N_CORES))],
                        ins=[cin.opt()], outs=[cout.opt()])
                    gd = p_sm.tile([P, GB], F32, tag="gd", name="gd")
                    nc.sync.dma_start(gd[:], cout[:])
                    rd = p_sm.tile([P, GB], F32, tag="rd", name="rd")
                    nc.vector.reciprocal(rd[:], gd[:])

                    for bl in range(GB):
                        bt = g * GB + bl
                        # normalization scale on the scalar engine — the
                        # vector engine is running the FIND_INDEX8 scans
                        nc.scalar.mul(exp_t[bl][:], exp_t[bl][:],
                                      rd[:, bl:bl + 1])
                        nc.sync.dma_start(act[bt * P:(bt + 1) * P, :],
                                          exp_t[bl][:])

    nc.compile()
    _CACHE["nc"] = nc
    return nc


def _split_bf16(x):
    hi = x.astype(ml_dtypes.bfloat16)
    lo = (x - hi.astype(np.float32)).astype(ml_dtypes.bfloat16)
    return np.ascontiguousarray(hi), np.ascontiguousarray(lo)


def kernel(features, W, b, prototypes, _run_kwargs=None):
    nc = _build()

    featT = np.asarray(features, dtype=np.float32).T
    WT = np.asarray(W, dtype=np.float32).T
    bvec = np.asarray(b, dtype=np.float32)
    prototypes = np.asarray(prototypes, dtype=np.float32)

    fh, fl = _split_bf16(featT)
    wh, wl = _split_bf16(WT)

    in_maps = []
    for c in range(N_CORES):
        ph, pl = _split_bf16(prototypes[c * KS:(c + 1) * KS].T)
        in_maps.append({"featT_hi": fh, "featT_lo": fl,
                        "WT_hi": wh, "WT_lo": wl, "bvec": bvec,
                        "protT_hi": ph, "protT_lo": pl})

    res = bass_utils.run_bass_kernel_spmd(
        nc, in_maps, core_ids=list(range(N_CORES)), **(_run_kwargs or {}))
    if _run_kwargs:
        _CACHE["last_result"] = res

    act = np.concatenate([res.results[c]["act"] for c in range(N_CORES)],
                         axis=1)
    lmax = np.stack([res.results[c]["lmax"][:, 0] for c in range(N_CORES)])
    lidx = np.stack([res.results[c]["lidx"][:, 0] for c in range(N_CORES)])
    best_core = np.argmax(lmax, axis=0)                       # [B]
    rows = np.arange(B)
    best_idx = (best_core * KS + lidx[best_core, rows]).astype(np.int32)
    return act, best_idx
